# revision 15
# baseline (speedup 1.0000x reference)
"""Causal multi-head self-attention with RoPE on 8 Trainium2 NeuronCores.

Model: B=2, S=2048, d_model=2048, H=16 heads, dk=128, fp32 I/O.

Sharding: tensor-parallel heads -> AllToAll -> row-parallel o_proj
(core c owns heads {2c, 2c+1}; after attention, two AllToAlls reshard the
head outputs so core j holds all 2048 head-dims for its 512 (batch,seq)
rows; each core then computes its row-block of the output projection).

v2 schedule: attention is interleaved with the QKV projections at seq-block
granularity. A q-tile slice t of batch b only needs blocks <= t of that
batch, so:
  unit k (k=0..3):  QKV chains for b0 block k, then attention slice
                    (head0, b0, t=k).
  unit k (k=4..7):  QKV chains for b1 block k-4, then slices
                    (head1, b0, t=k-4) and (head0, b1, t=k-4).
  tail:             AllToAll#0 (head0, overlaps the rest), slices
                    (head1, b1, t=0..3), AllToAll#1, then the output
                    projection in two waves (even heads from A2A#0 while
                    A2A#1 is in flight, odd heads on top).
This hides the scalar-engine exp sweeps (the attention bottleneck) under
the PE-bound projection phase and keeps the PE dense throughout.

Other changes vs v1: exact-causal slicing of score/exp/PV/es work (the
diagonal q-tile's dead columns are never computed), PSUM->SBUF evacuations
moved to the idle scalar engine, RoPE elementwise ops in bf16 (2x DVE
mode), trig tables in bf16 (half the DMA), first x-block DMA split into
chunks so the PE starts ~15us earlier.
"""

import math
from contextlib import ExitStack

import numpy as np
import ml_dtypes

import concourse.bass as bass
import concourse.tile as tile
import concourse.mybir as mybir
from concourse import bacc
from concourse import bass_utils

B = 2
S = 2048
D = 2048
H = 16
DK = 128
THETA = 10000.0
N_CORES = 8
HPC = H // N_CORES            # heads per core = 2
DPC = HPC * DK                # head dims per core = 256
ROWS = B * S                  # 4096 flattened rows
RPC = ROWS // N_CORES         # output rows per core = 512
SB = 512                      # seq block for projections
NSB = ROWS // SB              # 8 seq blocks (0-3 batch 0, 4-7 batch 1)
KC = 16                       # contraction chunks of 128 over D
QT = 512                      # q tile width in attention
NQT = S // QT                 # 4 q tiles per (b, h)
NKT = S // 128                # 16 k chunks per (b, h)
NJT = QT // 128               # 4 k chunks per q tile

BF16 = mybir.dt.bfloat16
F32 = mybir.dt.float32
MUL = None  # set below
ADD = None

_COMPILED = None


def _build():
    nc = bacc.Bacc("TRN2", target_bir_lowering=False, debug=False,
                   enable_asserts=False, num_devices=N_CORES)
    mul = mybir.AluOpType.mult
    add = mybir.AluOpType.add

    xT = nc.dram_tensor("xT", [NSB, 128, KC, SB], BF16, kind="ExternalInput")
    w3T = nc.dram_tensor("w3T", [3, HPC, 128, KC, 128], BF16,
                         kind="ExternalInput")
    woT = nc.dram_tensor("woT", [D, D], BF16, kind="ExternalInput")
    trig = nc.dram_tensor("trig", [2, B, 128, S], BF16, kind="ExternalInput")
    tri = nc.dram_tensor("tri", [128, 128], BF16, kind="ExternalInput")
    perm = nc.dram_tensor("perm", [128, 128], BF16, kind="ExternalInput")
    ones = nc.dram_tensor("ones", [128, 1], BF16, kind="ExternalInput")
    ident = nc.dram_tensor("ident", [128, 128], BF16, kind="ExternalInput")
    y_out = nc.dram_tensor("y", [RPC, D], F32, kind="ExternalOutput")

    scale = 1.0 / math.sqrt(DK)

    with tile.TileContext(nc) as tc, ExitStack() as outer:
        ccpool = outer.enter_context(
            tc.tile_pool(name="cc", bufs=1, space="DRAM"))
        cc_in = [ccpool.tile([N_CORES, 128, RPC], BF16, name=f"cc_in{oc}")
                 for oc in range(HPC)]
        cc_out = [ccpool.tile([N_CORES, 128, RPC], BF16, name=f"cc_out{oc}")
                  for oc in range(HPC)]

        consts = outer.enter_context(tc.tile_pool(name="consts", bufs=1))
        perm_sb = consts.tile([128, 128], BF16, name="perm_sb")
        ones_sb = consts.tile([128, 1], BF16, name="ones_sb")
        tri_sb = consts.tile([128, 128], BF16, name="tri_sb")
        ident_sb = consts.tile([128, 128], BF16, name="ident_sb")
        dummy_sb = consts.tile([1, 1], F32, name="dummy_sb")

        qk_pool = outer.enter_context(tc.tile_pool(name="qk", bufs=1))
        qT_sb = [[qk_pool.tile([128, S], BF16, name=f"q{o}_{b}_sb")
                  for b in range(B)] for o in range(HPC)]
        kT_sb = [[qk_pool.tile([128, S], BF16, name=f"k{o}_{b}_sb")
                  for b in range(B)] for o in range(HPC)]
        vtiles = outer.enter_context(tc.tile_pool(name="vtiles", bufs=1))
        v_sb = {}
        for b in range(B):
            for oc in range(HPC):
                for j in range(NKT):
                    v_sb[(b, oc, j)] = vtiles.tile(
                        [128, 128], BF16, name=f"v_{b}_{oc}_{j}")
        tpool = outer.enter_context(tc.tile_pool(name="trig", bufs=1))
        trig_sb = {}
        for b in range(B):
            for kind in range(2):
                trig_sb[(kind, b)] = tpool.tile(
                    [128, S], BF16, name=f"trig{kind}{b}")

        # attention-side SBUF pools (live across units + tail)
        epool = outer.enter_context(tc.tile_pool(name="E", bufs=8))
        espool = outer.enter_context(tc.tile_pool(name="esum", bufs=2))
        rpool = outer.enter_context(tc.tile_pool(name="recip", bufs=2))
        bpool = outer.enter_context(tc.tile_pool(name="bcast", bufs=2))
        apool = outer.enter_context(tc.tile_pool(name="attn", bufs=3))

        # o_proj inputs + weights
        atpool = outer.enter_context(tc.tile_pool(name="attnT", bufs=1))
        at_sb = [atpool.tile([128, RPC], BF16, name=f"at_{j2}")
                 for j2 in range(KC)]
        # even-head o_proj weights; odd-head tiles get their own pool created
        # after the QKV pools close (so the reservations never coexist)
        wopool_e = outer.enter_context(tc.tile_pool(name="woTe", bufs=8))

        # attention PSUM pools (live across units + tail): 4 banks.
        # Closed before the odd o_proj wave so its banks can be reused.
        p2 = outer.enter_context(ExitStack())
        sc_ps = p2.enter_context(
            tc.tile_pool(name="sc_psum", bufs=2, space="PSUM", side="right"))
        den_ps = p2.enter_context(
            tc.tile_pool(name="den_psum", bufs=1, space="PSUM", side="right"))
        out_ps = p2.enter_context(
            tc.tile_pool(name="out_psum", bufs=1, space="PSUM", side="right"))

        wo_sb = {}

        def load_wo(j2, pool):
            wo_sb[j2] = pool.tile([128, D], BF16, name="wo", tag="wo")
            nc.scalar.dma_start(
                wo_sb[j2][:], woT.ap()[j2 * 128:(j2 + 1) * 128, :])

        def attn_slice(oc, b, t):
            """Score/softmax/PV for q-tile t of (head oc, batch b)."""
            qT = qT_sb[oc][b]
            kT = kT_sb[oc][b]
            jmax = t * NJT + NJT - 1
            es = espool.tile([128, QT], BF16, name="esum", tag="esum")
            op = out_ps.tile([128, QT], F32, name="outp", tag="outp")
            for j in range(jmax + 1):
                r = j - t * NJT
                lo = 128 * r if r > 0 else 0
                ps = sc_ps.tile([128, QT], F32, name="sc", tag="sc")
                nc.tensor.matmul(
                    ps[:, lo:QT],
                    kT[:, j * 128:(j + 1) * 128],
                    qT[:, t * QT + lo:(t + 1) * QT],
                    start=True, stop=True)
                e = epool.tile([128, QT], BF16, name="E", tag="E")
                nc.scalar.activation(
                    e[:, lo:QT], ps[:, lo:QT],
                    mybir.ActivationFunctionType.Exp, scale=scale)
                if r >= 0:
                    # triangular mask on the diagonal 128x128 block
                    nc.vector.tensor_tensor(
                        e[:, 128 * r:128 * (r + 1)],
                        e[:, 128 * r:128 * (r + 1)],
                        tri_sb[:], mul)
                if j == 0:
                    nc.vector.tensor_copy(es[:], e[:])
                else:
                    nc.vector.tensor_tensor(
                        es[:, lo:QT], es[:, lo:QT], e[:, lo:QT], add)
                nc.tensor.matmul(
                    op[:, lo:QT], v_sb[(b, oc, j)][:], e[:, lo:QT],
                    start=(j == 0), stop=(j == jmax))
            dp = den_ps.tile([1, QT], F32, name="den", tag="den")
            nc.tensor.matmul(dp[:], ones_sb[:], es[:], start=True, stop=True)
            rc = rpool.tile([1, QT], F32, name="recip")
            nc.vector.reciprocal_approx_fast(rc[:], dp[:])
            bc = bpool.tile([128, QT], F32, name="bcast")
            nc.gpsimd.partition_broadcast(bc[:], rc[:])
            at = apool.tile([128, QT], BF16, name="attn_sb")
            nc.vector.tensor_tensor(at[:], op[:], bc[:], mul)
            nc.sync.dma_start(cc_in[oc][b * NQT + t, :, :], at[:])

        def emit_collective(oc):
            nc.gpsimd.collective_compute(
                "AllToAll",
                mybir.AluOpType.bypass,
                replica_groups=[list(range(N_CORES))],
                ins=[cc_in[oc].opt()],
                outs=[cc_out[oc].opt()],
            )

        # ---- QKV + RoPE phase pools (freed after unit 7) ----
        with ExitStack() as p1:
            xpool = p1.enter_context(tc.tile_pool(name="xT", bufs=2))
            wpool = p1.enter_context(tc.tile_pool(name="w3", bufs=1))
            qraw_pool = p1.enter_context(tc.tile_pool(name="qraw", bufs=2))
            rtmp = p1.enter_context(tc.tile_pool(name="rtmp", bufs=2))
            vt_pool = p1.enter_context(tc.tile_pool(name="vtmp", bufs=2))
            ppool = p1.enter_context(
                tc.tile_pool(name="qkv_psum", bufs=2, space="PSUM"))
            spool = p1.enter_context(
                tc.tile_pool(name="swap_psum", bufs=1, space="PSUM"))
            vtps_pool = p1.enter_context(
                tc.tile_pool(name="vt_psum", bufs=1, space="PSUM"))

            w_sb = {}

            def load_w(t, oc, eng):
                w_t = wpool.tile([128, KC, 128], BF16, name=f"w_{t}_{oc}")
                eng.dma_start(w_t[:], w3T.ap()[t, oc])
                w_sb[(t, oc)] = w_t

            x_tiles = [xpool.tile([128, KC, SB], BF16, name="xt_t")
                       for _ in range(NSB)]

            def load_x(sb):
                # 4 chunks of 4 ic each, alternating queues
                for c in range(4):
                    eng = nc.sync if c % 2 == 0 else nc.scalar
                    eng.dma_start(
                        x_tiles[sb][:, 4 * c:4 * c + 4, :],
                        xT.ap()[sb][:, 4 * c:4 * c + 4, :])

            def load_trig(kind, b, eng):
                eng.dma_start(trig_sb[(kind, b)][:], trig.ap()[kind, b])

            # ---- preamble DMAs ----
            load_w(0, 0, nc.scalar)
            nc.scalar.dma_start(perm_sb[:], perm.ap())
            load_x(0)                               # sync+scalar chunks
            # warm the ACT exp table while the first chains run
            nc.scalar.activation(dummy_sb[:], perm_sb[0:1, 0:1],
                                 mybir.ActivationFunctionType.Exp)
            load_trig(0, 0, nc.scalar)
            load_trig(1, 0, nc.scalar)
            load_w(1, 0, nc.scalar)
            nc.scalar.dma_start(ident_sb[:], ident.ap())
            load_w(2, 0, nc.scalar)
            nc.scalar.dma_start(tri_sb[:], tri.ap())
            nc.scalar.dma_start(ones_sb[:], ones.ap())
            load_w(0, 1, nc.sync)
            load_w(1, 1, nc.sync)
            load_w(2, 1, nc.sync)

            def rope_unit(ps, t, oc, b, scol):
                qraw = qraw_pool.tile([128, SB], BF16, name="qraw")
                nc.scalar.copy(qraw[:], ps[:])
                sw = spool.tile([128, SB], F32, name="swap_ps")
                nc.tensor.matmul(sw[:], perm_sb[:], qraw[:],
                                 start=True, stop=True)
                t1 = rtmp.tile([128, SB], BF16, name="t1")
                nc.vector.tensor_tensor(
                    t1[:], qraw[:],
                    trig_sb[(0, b)][:, scol:scol + SB], mul)
                t2 = rtmp.tile([128, SB], BF16, name="t2")
                nc.vector.tensor_tensor(
                    t2[:], sw[:],
                    trig_sb[(1, b)][:, scol:scol + SB], mul)
                dst = (qT_sb if t == 0 else kT_sb)[oc][b]
                nc.vector.tensor_tensor(
                    dst[:, scol:scol + SB], t1[:], t2[:], add)

            def v_unit(ps, b, oc, jb):
                vtmp = vt_pool.tile([128, SB], BF16, name="vtmp")
                nc.scalar.copy(vtmp[:], ps[:])
                for jj in range(4):
                    j = jb * 4 + jj
                    vt_ps = vtps_pool.tile([128, 128], BF16, name="vt_ps")
                    nc.tensor.transpose(
                        vt_ps[:], vtmp[:, jj * 128:(jj + 1) * 128],
                        ident_sb[:])
                    nc.scalar.copy(v_sb[(b, oc, j)][:], vt_ps[:])

            def unit(sb):
                b = sb // (NSB // B)
                jb = sb % (NSB // B)
                scol = jb * SB
                if sb + 1 < NSB:
                    load_x(sb + 1)
                if sb == 1:
                    load_trig(0, 1, nc.scalar)
                    load_trig(1, 1, nc.scalar)
                if sb >= 4:
                    # even-head o_proj weights, 2 per unit
                    load_wo(2 * (2 * (sb - 4)), wopool_e)
                    load_wo(2 * (2 * (sb - 4) + 1), wopool_e)
                for t, oc in ((0, 0), (1, 0), (2, 0), (0, 1), (1, 1), (2, 1)):
                    ps = ppool.tile([128, SB], F32, name="qkv_ps")
                    for ic in range(KC):
                        nc.tensor.matmul(
                            ps[:], w_sb[(t, oc)][:, ic, :],
                            x_tiles[sb][:, ic, :],
                            start=(ic == 0), stop=(ic == KC - 1))
                    if t < 2:
                        rope_unit(ps, t, oc, b, scol)
                    else:
                        v_unit(ps, b, oc, jb)
                if b == 0:
                    attn_slice(0, 0, jb)
                else:
                    attn_slice(1, 0, jb)
                    attn_slice(0, 1, jb)

            for sb in range(NSB):
                unit(sb)

        # ---- tail: A2A#0 early, head1/b1 attention, A2A#1 ----
        # ye_ps takes the 4 PSUM banks just freed by the QKV pools, so the
        # even o_proj wave can run concurrently with the tail attention
        # (which holds sc/den/out).
        yepool = outer.enter_context(tc.tile_pool(name="ye", bufs=1))
        ye_ps = outer.enter_context(
            tc.tile_pool(name="ye_psum", bufs=4, space="PSUM"))
        wopool_o = outer.enter_context(tc.tile_pool(name="woTo", bufs=8))

        emit_collective(0)
        for c in range(N_CORES):
            nc.sync.dma_start(at_sb[2 * c][:], cc_out[0][c])
        for t in range(NQT):
            load_wo(2 * t + 1, wopool_o)          # odd o_proj weights
            load_wo(2 * t + 9, wopool_o)
            attn_slice(1, 1, t)
        emit_collective(1)
        for c in range(N_CORES):
            nc.sync.dma_start(at_sb[2 * c + 1][:], cc_out[1][c])

        # ---- output projection in two waves ----
        NOT = D // 512  # 4 output tiles of 512
        ye_sb = {}
        for qc in range(RPC // 128):
            yp = [ye_ps.tile([128, 512], F32, name="ye_ps", tag="yeps")
                  for _ in range(NOT)]
            for idx, j2 in enumerate(range(0, KC, 2)):
                for ot in range(NOT):
                    nc.tensor.matmul(
                        yp[ot][:],
                        at_sb[j2][:, qc * 128:(qc + 1) * 128],
                        wo_sb[j2][:, ot * 512:(ot + 1) * 512],
                        start=(idx == 0), stop=(idx == KC // 2 - 1))
            for ot in range(NOT):
                y_t = yepool.tile([128, 512], BF16, name=f"ye_{qc}_{ot}")
                nc.scalar.copy(y_t[:], yp[ot][:])
                ye_sb[(qc, ot)] = y_t

        p2.close()  # free attention PSUM banks for the odd wave
        with ExitStack() as p3b:
            ypool = p3b.enter_context(tc.tile_pool(name="y_sb", bufs=4))
            y_ps = p3b.enter_context(
                tc.tile_pool(name="y_psum", bufs=4, space="PSUM"))
            for qc in range(RPC // 128):
                yp = [y_ps.tile([128, 512], F32, name="y_ps", tag="yps")
                      for _ in range(NOT)]
                for idx, j2 in enumerate(range(1, KC, 2)):
                    for ot in range(NOT):
                        nc.tensor.matmul(
                            yp[ot][:],
                            at_sb[j2][:, qc * 128:(qc + 1) * 128],
                            wo_sb[j2][:, ot * 512:(ot + 1) * 512],
                            start=(idx == 0), stop=(idx == KC // 2 - 1))
                for ot in range(NOT):
                    y_t = ypool.tile([128, 512], F32, name="y_t")
                    nc.vector.tensor_tensor(
                        y_t[:], yp[ot][:], ye_sb[(qc, ot)][:], add)
                    nc.scalar.dma_start(
                        y_out.ap()[qc * 128:(qc + 1) * 128,
                                   ot * 512:(ot + 1) * 512], y_t[:])

    nc.compile()
    return nc


def _host_inputs(x, token_positions, Wq, Wk, Wv, Wo):
    x = np.asarray(x, dtype=np.float32)
    pos = np.asarray(token_positions)
    Wq = np.asarray(Wq, dtype=np.float32)
    Wk = np.asarray(Wk, dtype=np.float32)
    Wv = np.asarray(Wv, dtype=np.float32)
    Wo = np.asarray(Wo, dtype=np.float32)

    bf = ml_dtypes.bfloat16
    # x pre-tiled for the QKV rhs: (sb, p, ic, s) = x[sb*SB+s, ic*128+p]
    xT = np.ascontiguousarray(
        x.reshape(NSB, SB, KC, 128).transpose(0, 3, 2, 1)).astype(bf)
    woT = np.ascontiguousarray(Wo.T).astype(bf)

    inv_freq = (1.0 / (THETA ** (np.arange(0, DK, 2, dtype=np.float32) / DK)))
    ang = pos.astype(np.float32)[:, None, :] * inv_freq[None, :, None]
    cos = np.cos(ang)
    sin = np.sin(ang)
    cos_dup = np.repeat(cos, 2, axis=1)                     # (B, 128, S)
    sin_sgn = np.repeat(sin, 2, axis=1)
    sin_sgn[:, 0::2, :] *= -1.0
    trig = np.stack([cos_dup, sin_sgn]).astype(bf)

    perm = np.zeros((128, 128), np.float32)
    for m in range(128):
        perm[m ^ 1, m] = 1.0
    perm = perm.astype(bf)

    kk = np.arange(128)[:, None]
    qq = np.arange(128)[None, :]
    tri = (kk <= qq).astype(np.float32).astype(bf)

    ones = np.ones((128, 1), np.float32).astype(bf)
    ident = np.eye(128, dtype=np.float32).astype(bf)

    in_maps = []
    for c in range(N_CORES):
        sl = slice(c * DPC, (c + 1) * DPC)
        # (t, oc, p, ic, o) = W[c*DPC + oc*128 + o, ic*128 + p]
        w3T = np.stack([
            W[sl, :].reshape(HPC, 128, KC, 128).transpose(0, 3, 2, 1)
            for W in (Wq, Wk, Wv)
        ]).astype(bf)
        in_maps.append({
            "xT": xT, "w3T": w3T, "woT": woT, "trig": trig,
            "tri": tri, "perm": perm, "ones": ones, "ident": ident,
        })
    return in_maps


def kernel(x, token_positions, Wq, Wk, Wv, Wo, _trace=False):
    global _COMPILED
    if _COMPILED is None:
        _COMPILED = _build()
    nc = _COMPILED

    in_maps = _host_inputs(x, token_positions, Wq, Wk, Wv, Wo)
    res = bass_utils.run_bass_kernel_spmd(
        nc, in_maps, core_ids=list(range(N_CORES)), trace=_trace)

    out = np.empty((ROWS, D), np.float32)
    for c in range(N_CORES):
        out[c * RPC:(c + 1) * RPC, :] = res.results[c]["y"]
    out = out.reshape(B, S, D)
    if _trace:
        return out, res
    return out


# revision 25
# speedup vs baseline: 1.0102x; 1.0102x over previous
"""Causal multi-head self-attention with RoPE on 8 Trainium2 NeuronCores.

Model: B=2, S=2048, d_model=2048, H=16 heads, dk=128, fp32 I/O.

Sharding: tensor-parallel heads -> AllToAll -> row-parallel o_proj
(core c owns heads {2c, 2c+1}; after attention, two AllToAlls reshard the
head outputs so core j holds all 2048 head-dims for its 512 (batch,seq)
rows; each core then computes its row-block of the output projection).

v2 schedule: attention is interleaved with the QKV projections at seq-block
granularity. A q-tile slice t of batch b only needs blocks <= t of that
batch, so:
  unit k (k=0..3):  QKV chains for b0 block k, then attention slice
                    (head0, b0, t=k).
  unit k (k=4..7):  QKV chains for b1 block k-4, then slices
                    (head1, b0, t=k-4) and (head0, b1, t=k-4).
  tail:             AllToAll#0 (head0, overlaps the rest), slices
                    (head1, b1, t=0..3), AllToAll#1, then the output
                    projection in two waves (even heads from A2A#0 while
                    A2A#1 is in flight, odd heads on top).
This hides the scalar-engine exp sweeps (the attention bottleneck) under
the PE-bound projection phase and keeps the PE dense throughout.

Other changes vs v1: exact-causal slicing of score/exp/PV/es work (the
diagonal q-tile's dead columns are never computed), PSUM->SBUF evacuations
moved to the idle scalar engine, RoPE elementwise ops in bf16 (2x DVE
mode), trig tables in bf16 (half the DMA), first x-block DMA split into
chunks so the PE starts ~15us earlier.
"""

import math
from contextlib import ExitStack

import numpy as np
import ml_dtypes

import concourse.bass as bass
import concourse.tile as tile
import concourse.mybir as mybir
from concourse import bacc
from concourse import bass_utils

B = 2
S = 2048
D = 2048
H = 16
DK = 128
THETA = 10000.0
N_CORES = 8
HPC = H // N_CORES            # heads per core = 2
DPC = HPC * DK                # head dims per core = 256
ROWS = B * S                  # 4096 flattened rows
RPC = ROWS // N_CORES         # output rows per core = 512
SB = 512                      # seq block for projections
NSB = ROWS // SB              # 8 seq blocks (0-3 batch 0, 4-7 batch 1)
KC = 16                       # contraction chunks of 128 over D
QT = 512                      # q tile width in attention
NQT = S // QT                 # 4 q tiles per (b, h)
NKT = S // 128                # 16 k chunks per (b, h)
NJT = QT // 128               # 4 k chunks per q tile

BF16 = mybir.dt.bfloat16
F32 = mybir.dt.float32
MUL = None  # set below
ADD = None

_COMPILED = None


def _build():
    nc = bacc.Bacc("TRN2", target_bir_lowering=False, debug=False,
                   enable_asserts=False, num_devices=N_CORES)
    mul = mybir.AluOpType.mult
    add = mybir.AluOpType.add

    xT = nc.dram_tensor("xT", [NSB, 128, KC, SB], BF16, kind="ExternalInput")
    w3T = nc.dram_tensor("w3T", [3, HPC, 128, KC, 128], BF16,
                         kind="ExternalInput")
    woT = nc.dram_tensor("woT", [D, D], BF16, kind="ExternalInput")
    trig = nc.dram_tensor("trig", [2, B, 128, S], BF16, kind="ExternalInput")
    tri = nc.dram_tensor("tri", [128, 128], BF16, kind="ExternalInput")
    perm = nc.dram_tensor("perm", [128, 128], BF16, kind="ExternalInput")
    ones = nc.dram_tensor("ones", [128, 1], BF16, kind="ExternalInput")
    onesr = nc.dram_tensor("onesr", [1, 128], BF16, kind="ExternalInput")
    ident = nc.dram_tensor("ident", [128, 128], BF16, kind="ExternalInput")
    y_out = nc.dram_tensor("y", [RPC, D], F32, kind="ExternalOutput")

    scale = 1.0 / math.sqrt(DK)

    with tile.TileContext(nc) as tc, ExitStack() as outer:
        ccpool = outer.enter_context(
            tc.tile_pool(name="cc", bufs=1, space="DRAM"))
        cc_in = [ccpool.tile([N_CORES, 128, RPC], BF16, name=f"cc_in{oc}")
                 for oc in range(HPC)]
        cc_out = [ccpool.tile([N_CORES, 128, RPC], BF16, name=f"cc_out{oc}")
                  for oc in range(HPC)]

        consts = outer.enter_context(tc.tile_pool(name="consts", bufs=1))
        perm_sb = consts.tile([128, 128], BF16, name="perm_sb")
        ones_sb = consts.tile([128, 1], BF16, name="ones_sb")
        onesr_sb = consts.tile([1, 128], BF16, name="onesr_sb")
        tri_sb = consts.tile([128, 128], BF16, name="tri_sb")
        ident_sb = consts.tile([128, 128], BF16, name="ident_sb")
        dummy_sb = consts.tile([1, 1], F32, name="dummy_sb")

        qk_pool = outer.enter_context(tc.tile_pool(name="qk", bufs=1))
        qT_sb = [[qk_pool.tile([128, S], BF16, name=f"q{o}_{b}_sb")
                  for b in range(B)] for o in range(HPC)]
        kT_sb = [[qk_pool.tile([128, S], BF16, name=f"k{o}_{b}_sb")
                  for b in range(B)] for o in range(HPC)]
        vtiles = outer.enter_context(tc.tile_pool(name="vtiles", bufs=1))
        v_sb = {}
        for b in range(B):
            for oc in range(HPC):
                for j in range(NKT):
                    v_sb[(b, oc, j)] = vtiles.tile(
                        [128, 128], BF16, name=f"v_{b}_{oc}_{j}")
        tpool = outer.enter_context(tc.tile_pool(name="trig", bufs=1))
        trig_sb = {}
        for b in range(B):
            for kind in range(2):
                trig_sb[(kind, b)] = tpool.tile(
                    [128, S], BF16, name=f"trig{kind}{b}")

        # attention-side SBUF pools (live across units + tail)
        epool = outer.enter_context(tc.tile_pool(name="E", bufs=8))
        espool = outer.enter_context(tc.tile_pool(name="esum", bufs=2))
        rpool = outer.enter_context(tc.tile_pool(name="recip", bufs=2))
        bpool = outer.enter_context(tc.tile_pool(name="bcast", bufs=2))
        apool = outer.enter_context(tc.tile_pool(name="attn", bufs=3))

        # o_proj inputs + weights
        atpool = outer.enter_context(tc.tile_pool(name="attnT", bufs=1))
        at_sb = [atpool.tile([128, RPC], BF16, name=f"at_{j2}")
                 for j2 in range(KC)]
        # even-head o_proj weights; odd-head tiles get their own pool created
        # after the QKV pools close (so the reservations never coexist)
        wopool_e = outer.enter_context(tc.tile_pool(name="woTe", bufs=8))

        # attention PSUM pools (live across units + tail): 4 banks.
        # Closed before the odd o_proj wave so its banks can be reused.
        p2 = outer.enter_context(ExitStack())
        sc_ps = p2.enter_context(
            tc.tile_pool(name="sc_psum", bufs=2, space="PSUM", side="right"))
        den_ps = p2.enter_context(
            tc.tile_pool(name="den_psum", bufs=1, space="PSUM", side="right"))
        out_ps = p2.enter_context(
            tc.tile_pool(name="out_psum", bufs=1, space="PSUM", side="right"))

        wo_sb = {}

        def load_wo(j2, pool):
            wo_sb[j2] = pool.tile([128, D], BF16, name="wo", tag="wo")
            nc.scalar.dma_start(
                wo_sb[j2][:], woT.ap()[j2 * 128:(j2 + 1) * 128, :])

        def attn_slice(oc, b, t, tail=False):
            """Score/softmax/PV for q-tile t of (head oc, batch b).

            tail=True avoids the gpsimd partition_broadcast: after the first
            AllToAll trigger is queued, gpsimd is blocked on its completion
            wait, so the broadcast runs as a K=1 matmul on the PE instead
            (and the PV accumulator is evacuated by the scalar engine so the
            DVE multiply has only one PSUM operand).
            """
            qT = qT_sb[oc][b]
            kT = kT_sb[oc][b]
            jmax = t * NJT + NJT - 1
            es = espool.tile([128, QT], BF16, name="esum", tag="esum")
            op = out_ps.tile([128, QT], F32, name="outp", tag="outp")
            for j in range(jmax + 1):
                r = j - t * NJT
                lo = 128 * r if r > 0 else 0
                ps = sc_ps.tile([128, QT], F32, name="sc", tag="sc")
                nc.tensor.matmul(
                    ps[:, lo:QT],
                    kT[:, j * 128:(j + 1) * 128],
                    qT[:, t * QT + lo:(t + 1) * QT],
                    start=True, stop=True)
                e = epool.tile([128, QT], BF16, name="E", tag="E")
                nc.scalar.activation(
                    e[:, lo:QT], ps[:, lo:QT],
                    mybir.ActivationFunctionType.Exp, scale=scale)
                if r >= 0:
                    # triangular mask on the diagonal 128x128 block
                    nc.vector.tensor_tensor(
                        e[:, 128 * r:128 * (r + 1)],
                        e[:, 128 * r:128 * (r + 1)],
                        tri_sb[:], mul)
                if j == 0:
                    nc.vector.tensor_copy(es[:], e[:])
                else:
                    nc.vector.tensor_tensor(
                        es[:, lo:QT], es[:, lo:QT], e[:, lo:QT], add)
                nc.tensor.matmul(
                    op[:, lo:QT], v_sb[(b, oc, j)][:], e[:, lo:QT],
                    start=(j == 0), stop=(j == jmax))
            dp = den_ps.tile([1, QT], F32, name="den", tag="den")
            nc.tensor.matmul(dp[:], ones_sb[:], es[:], start=True, stop=True)
            rc = rpool.tile([1, QT], F32, name="recip")
            nc.vector.reciprocal_approx_fast(rc[:], dp[:])
            at = apool.tile([128, QT], BF16, name="attn_sb", tag="attn_sb")
            if not tail:
                bc = bpool.tile([128, QT], F32, name="bcast")
                nc.gpsimd.partition_broadcast(bc[:], rc[:])
                nc.vector.tensor_tensor(at[:], op[:], bc[:], mul)
            else:
                rcb = rpool.tile([1, QT], BF16, name="recipb", tag="recipb")
                nc.vector.tensor_copy(rcb[:], rc[:])
                bcp = sc_ps.tile([128, QT], F32, name="bcp", tag="sc")
                nc.tensor.matmul(bcp[:], onesr_sb[:], rcb[:],
                                 start=True, stop=True)
                oph = apool.tile([128, QT], BF16, name="oph", tag="attn_sb")
                nc.scalar.copy(oph[:], op[:])
                nc.vector.tensor_tensor(at[:], oph[:], bcp[:], mul)
            nc.sync.dma_start(cc_in[oc][b * NQT + t, :, :], at[:])

        def emit_collective(oc):
            nc.gpsimd.collective_compute(
                "AllToAll",
                mybir.AluOpType.bypass,
                replica_groups=[list(range(N_CORES))],
                ins=[cc_in[oc].opt()],
                outs=[cc_out[oc].opt()],
            )

        # ---- QKV + RoPE phase pools (freed after unit 7) ----
        with ExitStack() as p1:
            xpool = p1.enter_context(tc.tile_pool(name="xT", bufs=2))
            wpool = p1.enter_context(tc.tile_pool(name="w3", bufs=1))
            qraw_pool = p1.enter_context(tc.tile_pool(name="qraw", bufs=2))
            rtmp = p1.enter_context(tc.tile_pool(name="rtmp", bufs=2))
            vt_pool = p1.enter_context(tc.tile_pool(name="vtmp", bufs=2))
            ppool = p1.enter_context(
                tc.tile_pool(name="qkv_psum", bufs=2, space="PSUM"))
            spool = p1.enter_context(
                tc.tile_pool(name="swap_psum", bufs=1, space="PSUM"))
            vtps_pool = p1.enter_context(
                tc.tile_pool(name="vt_psum", bufs=1, space="PSUM"))

            w_sb = {}

            def load_w(t, oc, eng):
                w_t = wpool.tile([128, KC, 128], BF16, name=f"w_{t}_{oc}")
                eng.dma_start(w_t[:], w3T.ap()[t, oc])
                w_sb[(t, oc)] = w_t

            x_tiles = [xpool.tile([128, KC, SB], BF16, name="xt_t")
                       for _ in range(NSB)]

            def load_x(sb):
                # 4 chunks of 4 ic each, alternating queues
                for c in range(4):
                    eng = nc.sync if c % 2 == 0 else nc.scalar
                    eng.dma_start(
                        x_tiles[sb][:, 4 * c:4 * c + 4, :],
                        xT.ap()[sb][:, 4 * c:4 * c + 4, :])

            def load_trig(kind, b, eng):
                eng.dma_start(trig_sb[(kind, b)][:], trig.ap()[kind, b])

            # ---- preamble DMAs ----
            # scalar queue: first chain's weights + x0 low chunks, then the
            # rope constants in need order; sync queue: x0 high chunks + the
            # oc=1 weights. Chain order is (q0,k0,v0,q1,k1,v1), ~4.2us each.
            load_w(0, 0, nc.scalar)
            nc.sync.dma_start(
                x_tiles[0][:, 8:12, :], xT.ap()[0][:, 8:12, :])
            nc.sync.dma_start(
                x_tiles[0][:, 12:16, :], xT.ap()[0][:, 12:16, :])
            nc.scalar.dma_start(
                x_tiles[0][:, 0:4, :], xT.ap()[0][:, 0:4, :])
            nc.scalar.dma_start(
                x_tiles[0][:, 4:8, :], xT.ap()[0][:, 4:8, :])
            nc.scalar.dma_start(perm_sb[:], perm.ap())
            # warm the ACT exp table while the first chains run
            nc.scalar.activation(dummy_sb[:], perm_sb[0:1, 0:1],
                                 mybir.ActivationFunctionType.Exp)
            load_w(1, 0, nc.scalar)
            load_trig(0, 0, nc.scalar)
            load_trig(1, 0, nc.scalar)
            nc.scalar.dma_start(ident_sb[:], ident.ap())
            load_w(2, 0, nc.scalar)
            nc.scalar.dma_start(tri_sb[:], tri.ap())
            nc.scalar.dma_start(ones_sb[:], ones.ap())
            nc.scalar.dma_start(onesr_sb[:], onesr.ap())
            load_w(0, 1, nc.sync)
            load_w(1, 1, nc.sync)
            load_w(2, 1, nc.sync)

            def rope_unit(ps, t, oc, b, scol):
                qraw = qraw_pool.tile([128, SB], BF16, name="qraw")
                nc.scalar.copy(qraw[:], ps[:])
                sw = spool.tile([128, SB], F32, name="swap_ps")
                nc.tensor.matmul(sw[:], perm_sb[:], qraw[:],
                                 start=True, stop=True)
                t1 = rtmp.tile([128, SB], BF16, name="t1")
                nc.vector.tensor_tensor(
                    t1[:], qraw[:],
                    trig_sb[(0, b)][:, scol:scol + SB], mul)
                t2 = rtmp.tile([128, SB], BF16, name="t2")
                nc.vector.tensor_tensor(
                    t2[:], sw[:],
                    trig_sb[(1, b)][:, scol:scol + SB], mul)
                dst = (qT_sb if t == 0 else kT_sb)[oc][b]
                nc.vector.tensor_tensor(
                    dst[:, scol:scol + SB], t1[:], t2[:], add)

            def v_unit(ps, b, oc, jb):
                vtmp = vt_pool.tile([128, SB], BF16, name="vtmp")
                nc.scalar.copy(vtmp[:], ps[:])
                for jj in range(4):
                    j = jb * 4 + jj
                    vt_ps = vtps_pool.tile([128, 128], BF16, name="vt_ps")
                    nc.tensor.transpose(
                        vt_ps[:], vtmp[:, jj * 128:(jj + 1) * 128],
                        ident_sb[:])
                    nc.scalar.copy(v_sb[(b, oc, j)][:], vt_ps[:])

            def unit(sb):
                b = sb // (NSB // B)
                jb = sb % (NSB // B)
                scol = jb * SB
                if sb + 1 < NSB:
                    load_x(sb + 1)
                if sb == 1:
                    load_trig(0, 1, nc.scalar)
                    load_trig(1, 1, nc.scalar)
                if sb >= 4:
                    # even-head o_proj weights, 2 per unit
                    load_wo(2 * (2 * (sb - 4)), wopool_e)
                    load_wo(2 * (2 * (sb - 4) + 1), wopool_e)
                for t, oc in ((0, 0), (1, 0), (2, 0), (0, 1), (1, 1), (2, 1)):
                    ps = ppool.tile([128, SB], F32, name="qkv_ps")
                    for ic in range(KC):
                        nc.tensor.matmul(
                            ps[:], w_sb[(t, oc)][:, ic, :],
                            x_tiles[sb][:, ic, :],
                            start=(ic == 0), stop=(ic == KC - 1))
                    if t < 2:
                        rope_unit(ps, t, oc, b, scol)
                    else:
                        v_unit(ps, b, oc, jb)
                if b == 0:
                    attn_slice(0, 0, jb)
                else:
                    attn_slice(1, 0, jb)
                    attn_slice(0, 1, jb)

            for sb in range(NSB):
                unit(sb)

        # ---- tail: A2A#0 early, head1/b1 attention, A2A#1 ----
        # ye_ps takes the 4 PSUM banks just freed by the QKV pools, so the
        # even o_proj wave can run concurrently with the tail attention
        # (which holds sc/den/out).
        yepool = outer.enter_context(tc.tile_pool(name="ye", bufs=1))
        ye_ps = outer.enter_context(
            tc.tile_pool(name="ye_psum", bufs=4, space="PSUM"))
        wopool_o = outer.enter_context(tc.tile_pool(name="woTo", bufs=8))

        emit_collective(0)
        for c in range(N_CORES):
            nc.sync.dma_start(at_sb[2 * c][:], cc_out[0][c])
        for t in range(NQT):
            load_wo(2 * t + 1, wopool_o)          # odd o_proj weights
            load_wo(2 * t + 9, wopool_o)
            attn_slice(1, 1, t, tail=True)
        emit_collective(1)
        for c in range(N_CORES):
            nc.sync.dma_start(at_sb[2 * c + 1][:], cc_out[1][c])

        # ---- output projection in two waves ----
        NOT = D // 512  # 4 output tiles of 512
        ye_sb = {}
        for qc in range(RPC // 128):
            yp = [ye_ps.tile([128, 512], F32, name="ye_ps", tag="yeps")
                  for _ in range(NOT)]
            for idx, j2 in enumerate(range(0, KC, 2)):
                for ot in range(NOT):
                    nc.tensor.matmul(
                        yp[ot][:],
                        at_sb[j2][:, qc * 128:(qc + 1) * 128],
                        wo_sb[j2][:, ot * 512:(ot + 1) * 512],
                        start=(idx == 0), stop=(idx == KC // 2 - 1))
            for ot in range(NOT):
                y_t = yepool.tile([128, 512], BF16, name=f"ye_{qc}_{ot}")
                nc.scalar.copy(y_t[:], yp[ot][:])
                ye_sb[(qc, ot)] = y_t

        p2.close()  # free attention PSUM banks for the odd wave
        with ExitStack() as p3b:
            ypool = p3b.enter_context(tc.tile_pool(name="y_sb", bufs=4))
            y_ps = p3b.enter_context(
                tc.tile_pool(name="y_psum", bufs=4, space="PSUM"))
            for qc in range(RPC // 128):
                yp = [y_ps.tile([128, 512], F32, name="y_ps", tag="yps")
                      for _ in range(NOT)]
                for idx, j2 in enumerate(range(1, KC, 2)):
                    for ot in range(NOT):
                        nc.tensor.matmul(
                            yp[ot][:],
                            at_sb[j2][:, qc * 128:(qc + 1) * 128],
                            wo_sb[j2][:, ot * 512:(ot + 1) * 512],
                            start=(idx == 0), stop=(idx == KC // 2 - 1))
                for ot in range(NOT):
                    y_t = ypool.tile([128, 512], F32, name="y_t")
                    nc.vector.tensor_tensor(
                        y_t[:], yp[ot][:], ye_sb[(qc, ot)][:], add)
                    nc.scalar.dma_start(
                        y_out.ap()[qc * 128:(qc + 1) * 128,
                                   ot * 512:(ot + 1) * 512], y_t[:])

    nc.compile()
    return nc


def _host_inputs(x, token_positions, Wq, Wk, Wv, Wo):
    x = np.asarray(x, dtype=np.float32)
    pos = np.asarray(token_positions)
    Wq = np.asarray(Wq, dtype=np.float32)
    Wk = np.asarray(Wk, dtype=np.float32)
    Wv = np.asarray(Wv, dtype=np.float32)
    Wo = np.asarray(Wo, dtype=np.float32)

    bf = ml_dtypes.bfloat16
    # x pre-tiled for the QKV rhs: (sb, p, ic, s) = x[sb*SB+s, ic*128+p]
    xT = np.ascontiguousarray(
        x.reshape(NSB, SB, KC, 128).transpose(0, 3, 2, 1)).astype(bf)
    woT = np.ascontiguousarray(Wo.T).astype(bf)

    inv_freq = (1.0 / (THETA ** (np.arange(0, DK, 2, dtype=np.float32) / DK)))
    ang = pos.astype(np.float32)[:, None, :] * inv_freq[None, :, None]
    cos = np.cos(ang)
    sin = np.sin(ang)
    cos_dup = np.repeat(cos, 2, axis=1)                     # (B, 128, S)
    sin_sgn = np.repeat(sin, 2, axis=1)
    sin_sgn[:, 0::2, :] *= -1.0
    trig = np.stack([cos_dup, sin_sgn]).astype(bf)

    perm = np.zeros((128, 128), np.float32)
    for m in range(128):
        perm[m ^ 1, m] = 1.0
    perm = perm.astype(bf)

    kk = np.arange(128)[:, None]
    qq = np.arange(128)[None, :]
    tri = (kk <= qq).astype(np.float32).astype(bf)

    ones = np.ones((128, 1), np.float32).astype(bf)
    onesr = np.ones((1, 128), np.float32).astype(bf)
    ident = np.eye(128, dtype=np.float32).astype(bf)

    in_maps = []
    for c in range(N_CORES):
        sl = slice(c * DPC, (c + 1) * DPC)
        # (t, oc, p, ic, o) = W[c*DPC + oc*128 + o, ic*128 + p]
        w3T = np.stack([
            W[sl, :].reshape(HPC, 128, KC, 128).transpose(0, 3, 2, 1)
            for W in (Wq, Wk, Wv)
        ]).astype(bf)
        in_maps.append({
            "xT": xT, "w3T": w3T, "woT": woT, "trig": trig,
            "tri": tri, "perm": perm, "ones": ones, "onesr": onesr,
            "ident": ident,
        })
    return in_maps


def kernel(x, token_positions, Wq, Wk, Wv, Wo, _trace=False):
    global _COMPILED
    if _COMPILED is None:
        _COMPILED = _build()
    nc = _COMPILED

    in_maps = _host_inputs(x, token_positions, Wq, Wk, Wv, Wo)
    res = bass_utils.run_bass_kernel_spmd(
        nc, in_maps, core_ids=list(range(N_CORES)), trace=_trace)

    out = np.empty((ROWS, D), np.float32)
    for c in range(N_CORES):
        out[c * RPC:(c + 1) * RPC, :] = res.results[c]["y"]
    out = out.reshape(B, S, D)
    if _trace:
        return out, res
    return out
